# revision 19
# baseline (speedup 1.0000x reference)
"""NeuralMMU Trainium2 kernel (v2: ACT-bound pipeline).

Per core: 131072 addrs, 64 iterations x 2048 addrs.

Engine plan per iteration t (steady state, ~1.9us period):
  ACT   Gelu(+b1): hpre slot(t) PSUM [128,2048] -> h(t) SBUF f32.
        One op per iter; this is the bottleneck engine (~1892 ns).
  PE    L2(t-1): 16 matmuls with SWAPPED operands: stationary lhsT =
        h(t-1)[:, 128c:128c+128] (f32, exact), moving rhs = W2 [128,26]
        f32 -> batch-major logits [128 batch, 26] written into the TAIL
        416 f32 of psum slot(t-1) (bank 3), which gelu(t-1) has already
        consumed.  26 cols * 4 cyc/row * 16 = 1664 cyc.
        L1(t+1): 4 bf16 matmuls k=96 (3-way bf16 split of W1, exact to
        ~2^-27) from host-prepared bf16 bit planes -> slot(t+1).
        Blocks g=0..2 issue early; block g=3 (tail bank) waits until the
        DVE threshold has read slot(t+1)'s previous logits.
  DVE   TT is_gt vs per-logit threshold vector (0.5 - b2[j], f32,
        partition-broadcast) -> bits bf16; TT mult by 2^(j%13) weight
        vector; tensor_reduce sum [128,16,2,13] -> packed lo/hi
        [128,32] f32 into an 8-iter accumulator.
  DMA   in: [96,4096] bf16 planes per 2 iters; out: [128,256] f32 per
        8 iters.  Host packs bit planes and combines lo+8192*hi.

PSUM: exactly 8 banks = 2 slots x [128,2048] f32; L2 output aliases the
tail of the slot (time-multiplexed with hpre data).

Numerics are f32-exact end-to-end except the 3-way-bf16 W1 split
(~2^-27) and the ACT Gelu LUT, identical to the f32 baseline (1/1M
mismatch there).
"""

import numpy as np
from contextlib import ExitStack

import concourse.bass as bass
import concourse.mybir as mybir
import concourse.tile as tile
from concourse import bacc, bass_utils

B = 1_048_576
NCORES = 8
PER = B // NCORES          # 131072 addrs per core
BLK = 512                  # addrs per L1 PE block
NBLK = 4                   # L1 blocks per iteration
CH = 128                   # addrs per L2 chunk (stationary width)
NCH = 16                   # L2 chunks per iteration
CHUNK = NBLK * BLK         # 2048 addrs per iteration
N_ITERS = PER // CHUNK     # 64
GIN = 2                    # iters per input DMA
GOUT = 8                   # iters per output DMA
NLOG = 26                  # logits per addr
LW = NCH * NLOG            # 416 logit cols per iteration
HLW = LW // 2              # 208: one thresh half

F32 = mybir.dt.float32
BF16 = mybir.dt.bfloat16
AF = mybir.ActivationFunctionType
ALU = mybir.AluOpType

# cst columns (f32): w1b 0:64 (bf16x128), b1c 64:65, w2f 65:91,
# wvec 91:299 (bf16 x416), thvec 299:715
CW1, CB1, CW2, CWV, CTH, CTOT = 0, 64, 65, 91, 299, 715


def build_nc(n_iters: int = N_ITERS, act=AF.Gelu) -> bass.Bass:
    nc = bacc.Bacc("TRN2")
    assert n_iters % GOUT == 0 and n_iters % GIN == 0

    bp = nc.dram_tensor("bp", [n_iters // GIN, 96, GIN * CHUNK], BF16,
                        kind="ExternalInput")
    cst_d = nc.dram_tensor("cst", [128, CTOT], F32, kind="ExternalInput")
    outp = nc.dram_tensor("outp", [n_iters // GOUT, 128, GOUT * 32], F32,
                          kind="ExternalOutput")

    with ExitStack() as ctx:
        tc = ctx.enter_context(tile.TileContext(nc))
        const = ctx.enter_context(tc.tile_pool(name="const", bufs=1))
        rpool = ctx.enter_context(tc.tile_pool(name="rp", bufs=3))
        ppool = ctx.enter_context(
            tc.tile_pool(name="ppool", bufs=1, space="PSUM"))
        hp = ctx.enter_context(tc.tile_pool(name="hp", bufs=2))
        bop = ctx.enter_context(tc.tile_pool(name="bop", bufs=2))
        bwp = ctx.enter_context(tc.tile_pool(name="bwp", bufs=2))
        pksp = ctx.enter_context(tc.tile_pool(name="pksp", bufs=2))

        # One persistent 8-bank psum tensor; all deps are subtile
        # (range-based), avoiding tile-granular pool-rotation WAR chains.
        PP = ppool.tile([128, 2 * CHUNK], F32, name="PP")

        cst = const.tile([128, CTOT], F32)
        nc.sync.dma_start(cst[:], cst_d[:])
        w1b = cst[:, CW1:CB1].bitcast(BF16)      # [128,128] bf16; rows 0-95
        b1c = cst[:, CB1:CW2]
        w2f = cst[:, CW2:CWV]                    # [128, 26] f32
        wv = cst[:, CWV:CTH].bitcast(BF16)       # [128, 416] bf16
        thv = cst[:, CTH:CTOT]                   # [128, 416] f32

        R = {}
        hs = {}
        bos = {}
        pks = None

        def half(t):
            return CHUNK * (t % 2)

        def load_input(g):
            if g < n_iters // GIN and g not in R:
                r = rpool.tile([96, GIN * CHUNK], BF16, name="r")
                nc.sync.dma_start(r[:], bp[g])
                R[g] = r

        def l1(t, segs):
            """segs: list of (c0, c1) col ranges within the iteration.
            Cols 0:208 / 208:416 hold the previous era's logits, so those
            small segments are issued last, each gated only on its own
            thresh half (A / B) having read them."""
            if t >= n_iters:
                return
            r = R[t // GIN]
            for s0, s1 in segs:
                c0 = CHUNK * (t % GIN) + s0
                nc.tensor.matmul(
                    PP[:, half(t) + s0:half(t) + s1],
                    w1b[0:96, :],
                    r[0:96, c0:c0 + (s1 - s0)],
                    start=True, stop=True, tile_position=(0, 0),
                )

        L1_BIG = [(416, 512), (512, 1024), (1024, 1536), (1536, 2048)]
        L1_A = [(0, HLW)]
        L1_B = [(HLW, LW)]

        def gelu(t):
            h = hp.tile([128, CHUNK], F32, name="h")
            nc.scalar.activation(h[:], PP[:, half(t):half(t) + CHUNK], act,
                                 bias=b1c, scale=1.0)
            hs[t] = h

        def l2(t, chunks):
            h = hs[t]
            for c in chunks:
                nc.tensor.matmul(
                    PP[:, half(t) + NLOG * c:half(t) + NLOG * (c + 1)],
                    h[:, CH * c:CH * (c + 1)],
                    w2f[:],
                    start=True, stop=True, tile_position=(0, 0),
                )

        def thresh(t, hi):
            """hi=0: logit cols 0:208 (chunks 0-7); hi=1: 208:416."""
            if not hi:
                bos[t] = bop.tile([128, LW], BF16, name="bo")
            o = HLW * hi
            nc.vector.tensor_tensor(
                bos[t][:, o:o + HLW],
                PP[:, half(t) + o:half(t) + o + HLW],
                thv[:, o:o + HLW],
                op=ALU.is_gt)

        def pack(t):
            nonlocal pks
            bw = bwp.tile([128, LW], BF16, name="bw")
            nc.gpsimd.tensor_tensor(bw[:], bos.pop(t)[:], wv, op=ALU.mult)
            if t % GOUT == 0:
                pks = pksp.tile([128, GOUT * 32], F32, name="pks")
            nc.vector.tensor_reduce(
                pks[:, 32 * (t % GOUT):32 * (t % GOUT + 1)],
                bw[:].rearrange("p (g x) -> p g x", x=13),
                axis=mybir.AxisListType.X,
                op=ALU.add,
            )
            if t % GOUT == GOUT - 1:
                nc.sync.dma_start(outp[t // GOUT], pks[:])

        # Prologue: planes for iters 0-3, L1(0).
        load_input(0)
        load_input(1)
        l1(0, L1_BIG + L1_A + L1_B)

        for t in range(n_iters):
            gelu(t)
            if t >= 1:
                l2(t - 1, range(NCH // 2))
                thresh(t - 1, 0)
                l2(t - 1, range(NCH // 2, NCH))
                thresh(t - 1, 1)
                hs.pop(t - 1)
            if t % GIN == 0:
                load_input(t // GIN + 2)
            l1(t + 1, L1_BIG)
            if t >= 1:
                pack(t - 1)
            l1(t + 1, L1_A)
            l1(t + 1, L1_B)

        l2(n_iters - 1, range(NCH))
        hs.pop(n_iters - 1)
        thresh(n_iters - 1, 0)
        thresh(n_iters - 1, 1)
        pack(n_iters - 1)

    return nc


def make_const_inputs(W1, b1, W2, b2):
    import ml_dtypes

    w1 = np.ascontiguousarray(W1[0:32, :], dtype=np.float32)
    hi = w1.astype(ml_dtypes.bfloat16)
    mid = (w1 - hi.astype(np.float32)).astype(ml_dtypes.bfloat16)
    lo = (w1 - hi.astype(np.float32) - mid.astype(np.float32)).astype(
        ml_dtypes.bfloat16
    )
    w1b = np.zeros((128, 128), dtype=ml_dtypes.bfloat16)
    w1b[0:32] = hi
    w1b[32:64] = mid
    w1b[64:96] = lo

    cst = np.zeros((128, CTOT), dtype=np.float32)
    cst[:, CW1:CB1] = np.ascontiguousarray(w1b).view(np.float32)
    cst[:, CB1] = np.asarray(b1, dtype=np.float32)
    cst[:, CW2:CWV] = np.asarray(W2[:, :NLOG], dtype=np.float32)
    wvec = np.tile(
        np.concatenate([2.0 ** np.arange(13), 2.0 ** np.arange(13)]), NCH
    ).astype(ml_dtypes.bfloat16)        # [416]
    cst[:, CWV:CTH] = np.ascontiguousarray(wvec).view(np.float32)[None, :]
    thvec = np.tile(0.5 - np.asarray(b2[:NLOG], dtype=np.float32), NCH)
    cst[:, CTH:CTOT] = thvec[None, :]
    return {"cst": cst}


def make_bit_planes(virtual_addr, n_iters: int = N_ITERS):
    """Per-core [n_iters//GIN, 96, GIN*2048] bf16 0/1 bit planes.

    Partition 32s + k (s = 0..2 replication) of DMA group tt, col
    j*2048 + n = bit k of addr (GIN*tt + j)*2048 + n.
    """
    import ml_dtypes

    va32 = np.asarray(virtual_addr).astype(np.uint32)
    per = n_iters * CHUNK
    ncores = va32.size // per
    out = []
    for c in range(ncores):
        seg = va32[c * per:(c + 1) * per]
        byt = seg.view(np.uint8).reshape(n_iters // GIN, GIN * CHUNK, 4)
        bits = np.unpackbits(byt, axis=-1, bitorder="little")
        # (tt, n, k) -> (tt, k, n)
        pl = bits.transpose(0, 2, 1)
        pl3 = np.concatenate([pl, pl, pl], axis=1).astype(ml_dtypes.bfloat16)
        out.append(np.ascontiguousarray(pl3))
    return out


def combine_output(o, n_iters: int = N_ITERS):
    """[n_iters//GOUT, 128, GOUT*32] f32 -> [per] int64.

    col 32*ts + 2*c + half: lo/hi 13-bit halves of chunk c, iter
    GOUT*tt + ts; addr = CHUNK*t + CH*c + p.
    """
    arr = np.asarray(o, dtype=np.int64).reshape(
        n_iters // GOUT, 128, GOUT, NCH, 2)
    lo = arr[..., 0]                     # [tt, p, ts, c]
    hi = arr[..., 1]
    val = lo + 8192 * hi                 # [tt, p, ts, c]
    return val.transpose(0, 2, 3, 1).reshape(-1)


_NC_CACHE = {}
TRACE = False
LAST_RES = None


def kernel(virtual_addr, W1, b1, W2, b2):
    global LAST_RES
    if "nc" not in _NC_CACHE:
        nc = build_nc(N_ITERS)
        nc.finalize()
        _NC_CACHE["nc"] = nc
    nc = _NC_CACHE["nc"]

    consts = make_const_inputs(W1, b1, W2, b2)
    planes = make_bit_planes(virtual_addr, N_ITERS)
    in_maps = [{"bp": planes[c], **consts} for c in range(NCORES)]

    res = bass_utils.run_bass_kernel_spmd(
        nc, in_maps, list(range(NCORES)), trace=TRACE
    )
    LAST_RES = res

    outs = [combine_output(res.results[c]["outp"]) for c in range(NCORES)]
    return np.concatenate(outs)


# revision 22
# speedup vs baseline: 1.2637x; 1.2637x over previous
"""NeuralMMU Trainium2 kernel (v2: ACT-bound pipeline).

Per core: 131072 addrs, 64 iterations x 2048 addrs.

Engine plan per iteration t (steady state, ~1.9us period):
  ACT   Gelu(+b1): hpre slot(t) PSUM [128,2048] -> h(t) SBUF f32.
        One op per iter; this is the bottleneck engine (~1892 ns).
  PE    L2(t-1): 16 matmuls with SWAPPED operands: stationary lhsT =
        h(t-1)[:, 128c:128c+128] (f32, exact), moving rhs = W2 [128,26]
        f32 -> batch-major logits [128 batch, 26] written into the TAIL
        416 f32 of psum slot(t-1) (bank 3), which gelu(t-1) has already
        consumed.  26 cols * 4 cyc/row * 16 = 1664 cyc.
        L1(t+1): 4 bf16 matmuls k=96 (3-way bf16 split of W1, exact to
        ~2^-27) from host-prepared bf16 bit planes -> slot(t+1).
        Blocks g=0..2 issue early; block g=3 (tail bank) waits until the
        DVE threshold has read slot(t+1)'s previous logits.
  DVE   TT is_gt vs per-logit threshold vector (0.5 - b2[j], f32,
        partition-broadcast) -> bits bf16; TT mult by 2^(j%13) weight
        vector; tensor_reduce sum [128,16,2,13] -> packed lo/hi
        [128,32] f32 into an 8-iter accumulator.
  DMA   in: [96,4096] bf16 planes per 2 iters; out: [128,256] f32 per
        8 iters.  Host packs bit planes and combines lo+8192*hi.

PSUM: exactly 8 banks = 2 slots x [128,2048] f32; L2 output aliases the
tail of the slot (time-multiplexed with hpre data).

Numerics are f32-exact end-to-end except the 3-way-bf16 W1 split
(~2^-27) and the ACT Gelu LUT, identical to the f32 baseline (1/1M
mismatch there).
"""

import numpy as np
from contextlib import ExitStack

import concourse.bass as bass
import concourse.mybir as mybir
import concourse.tile as tile
from concourse import bacc, bass_utils

B = 1_048_576
NCORES = 8
PER = B // NCORES          # 131072 addrs per core
BLK = 512                  # addrs per L1 PE block
NBLK = 4                   # L1 blocks per iteration
CH = 128                   # addrs per L2 chunk (stationary width)
NCH = 16                   # L2 chunks per iteration
CHUNK = NBLK * BLK         # 2048 addrs per iteration
N_ITERS = PER // CHUNK     # 64
GIN = 2                    # iters per input DMA
GOUT = 8                   # iters per output DMA
NLOG = 26                  # logits per addr
LW = NCH * NLOG            # 416 logit cols per iteration
HLW = LW // 2              # 208: one thresh half

F32 = mybir.dt.float32
BF16 = mybir.dt.bfloat16
AF = mybir.ActivationFunctionType
ALU = mybir.AluOpType

# cst columns (f32): w1b 0:64 (bf16x128), b1c 64:65, w2f 65:91,
# wvec 91:299 (bf16 x416), thvec 299:715
CW1, CB1, CW2, CWV, CTH, CTOT = 0, 64, 65, 91, 299, 715


def build_nc(n_iters: int = N_ITERS, act=AF.Gelu) -> bass.Bass:
    nc = bacc.Bacc("TRN2")
    assert n_iters % GOUT == 0 and n_iters % GIN == 0

    bp = nc.dram_tensor("bp", [n_iters // GIN, 96, GIN * CHUNK], BF16,
                        kind="ExternalInput")
    cst_d = nc.dram_tensor("cst", [128, CTOT], F32, kind="ExternalInput")
    outp = nc.dram_tensor("outp", [n_iters // GOUT, 128, GOUT * 32], F32,
                          kind="ExternalOutput")

    with ExitStack() as ctx:
        tc = ctx.enter_context(tile.TileContext(nc))
        const = ctx.enter_context(tc.tile_pool(name="const", bufs=1))
        rpool = ctx.enter_context(tc.tile_pool(name="rp", bufs=3))
        ppool = ctx.enter_context(
            tc.tile_pool(name="ppool", bufs=1, space="PSUM"))
        hp = ctx.enter_context(tc.tile_pool(name="hp", bufs=2))
        bop = ctx.enter_context(tc.tile_pool(name="bop", bufs=2))
        bwp = ctx.enter_context(tc.tile_pool(name="bwp", bufs=2))
        pksp = ctx.enter_context(tc.tile_pool(name="pksp", bufs=2))

        # One persistent 8-bank psum tensor; all deps are subtile
        # (range-based), avoiding tile-granular pool-rotation WAR chains.
        PP = ppool.tile([128, 2 * CHUNK], F32, name="PP")

        cst = const.tile([128, CTOT], F32)
        nc.sync.dma_start(cst[:], cst_d[:])
        w1b = cst[:, CW1:CB1].bitcast(BF16)      # [128,128] bf16; rows 0-95
        b1c = cst[:, CB1:CW2]
        w2f = cst[:, CW2:CWV]                    # [128, 26] f32
        wv = cst[:, CWV:CTH].bitcast(BF16)       # [128, 416] bf16
        thv = cst[:, CTH:CTOT]                   # [128, 416] f32

        R = {}
        hs = {}
        bos = {}
        pks = None

        def half(t):
            return CHUNK * (t % 2)

        def load_input(g):
            if g < n_iters // GIN and g not in R:
                r = rpool.tile([96, GIN * CHUNK], BF16, name="r")
                nc.sync.dma_start(r[:], bp[g])
                R[g] = r

        def l1(t, blocks):
            """blocks: bank indices 0..3 (512 cols each).  Banks 0 and 1
            hold the previous era's logits in their head cols, so each is
            issued after its thresh (A / B) has read them; banks 2-3 are
            free early."""
            if t >= n_iters:
                return
            r = R[t // GIN]
            for g in blocks:
                c0 = CHUNK * (t % GIN) + BLK * g
                nc.tensor.matmul(
                    PP[:, half(t) + BLK * g:half(t) + BLK * (g + 1)],
                    w1b[0:96, :],
                    r[0:96, c0:c0 + BLK],
                    start=True, stop=True, tile_position=(0, 0),
                )

        def gelu(t):
            h = hp.tile([128, CHUNK], F32, name="h")
            nc.scalar.activation(h[:], PP[:, half(t):half(t) + CHUNK], act,
                                 bias=b1c, scale=1.0)
            hs[t] = h

        NA = 6                     # chunks in bank-0 logit home
        AW = NA * NLOG             # 156 cols
        BW = LW - AW               # 260 cols (chunks 6-15 in bank-1 home)

        def l2col(c):
            """psum col of chunk c's logits: bank0 head for c<NA, bank1
            head (col 512+) for c>=NA."""
            return NLOG * c if c < NA else BLK + NLOG * (c - NA)

        def l2(t, chunks):
            h = hs[t]
            for c in chunks:
                o = l2col(c)
                nc.tensor.matmul(
                    PP[:, half(t) + o:half(t) + o + NLOG],
                    h[:, CH * c:CH * (c + 1)],
                    w2f[:],
                    start=True, stop=True, tile_position=(0, 0),
                )

        def thresh(t, hi):
            """hi=0: bank0 logits (chunks 0..NA-1); hi=1: bank1 logits."""
            if not hi:
                bos[t] = bop.tile([128, LW], BF16, name="bo")
                nc.vector.tensor_tensor(
                    bos[t][:, 0:AW], PP[:, half(t):half(t) + AW],
                    thv[:, 0:AW], op=ALU.is_gt)
            else:
                nc.vector.tensor_tensor(
                    bos[t][:, AW:LW], PP[:, half(t) + BLK:half(t) + BLK + BW],
                    thv[:, 0:BW], op=ALU.is_gt)

        def pack(t):
            nonlocal pks
            bw = bwp.tile([128, LW], BF16, name="bw")
            nc.gpsimd.tensor_tensor(bw[:], bos.pop(t)[:], wv, op=ALU.mult)
            if t % GOUT == 0:
                pks = pksp.tile([128, GOUT * 32], F32, name="pks")
            nc.vector.tensor_reduce(
                pks[:, 32 * (t % GOUT):32 * (t % GOUT + 1)],
                bw[:].rearrange("p (g x) -> p g x", x=13),
                axis=mybir.AxisListType.X,
                op=ALU.add,
            )
            if t % GOUT == GOUT - 1:
                nc.sync.dma_start(outp[t // GOUT], pks[:])

        # Prologue: planes for iters 0-3, L1(0).
        load_input(0)
        load_input(1)
        l1(0, (2, 3, 0, 1))

        for t in range(n_iters):
            gelu(t)
            if t >= 1:
                l2(t - 1, range(NA))
                thresh(t - 1, 0)
                l2(t - 1, range(NA, NCH))
                thresh(t - 1, 1)
                hs.pop(t - 1)
            if t % GIN == 0:
                load_input(t // GIN + 2)
            l1(t + 1, (2, 3))
            if t >= 2:
                pack(t - 2)
            l1(t + 1, (0,))
            l1(t + 1, (1,))

        l2(n_iters - 1, range(NCH))
        hs.pop(n_iters - 1)
        thresh(n_iters - 1, 0)
        thresh(n_iters - 1, 1)
        pack(n_iters - 2)
        pack(n_iters - 1)

    return nc


def make_const_inputs(W1, b1, W2, b2):
    import ml_dtypes

    w1 = np.ascontiguousarray(W1[0:32, :], dtype=np.float32)
    hi = w1.astype(ml_dtypes.bfloat16)
    mid = (w1 - hi.astype(np.float32)).astype(ml_dtypes.bfloat16)
    lo = (w1 - hi.astype(np.float32) - mid.astype(np.float32)).astype(
        ml_dtypes.bfloat16
    )
    w1b = np.zeros((128, 128), dtype=ml_dtypes.bfloat16)
    w1b[0:32] = hi
    w1b[32:64] = mid
    w1b[64:96] = lo

    cst = np.zeros((128, CTOT), dtype=np.float32)
    cst[:, CW1:CB1] = np.ascontiguousarray(w1b).view(np.float32)
    cst[:, CB1] = np.asarray(b1, dtype=np.float32)
    cst[:, CW2:CWV] = np.asarray(W2[:, :NLOG], dtype=np.float32)
    wvec = np.tile(
        np.concatenate([2.0 ** np.arange(13), 2.0 ** np.arange(13)]), NCH
    ).astype(ml_dtypes.bfloat16)        # [416]
    cst[:, CWV:CTH] = np.ascontiguousarray(wvec).view(np.float32)[None, :]
    thvec = np.tile(0.5 - np.asarray(b2[:NLOG], dtype=np.float32), NCH)
    cst[:, CTH:CTOT] = thvec[None, :]
    return {"cst": cst}


def make_bit_planes(virtual_addr, n_iters: int = N_ITERS):
    """Per-core [n_iters//GIN, 96, GIN*2048] bf16 0/1 bit planes.

    Partition 32s + k (s = 0..2 replication) of DMA group tt, col
    j*2048 + n = bit k of addr (GIN*tt + j)*2048 + n.
    """
    import ml_dtypes

    va32 = np.asarray(virtual_addr).astype(np.uint32)
    per = n_iters * CHUNK
    ncores = va32.size // per
    out = []
    for c in range(ncores):
        seg = va32[c * per:(c + 1) * per]
        byt = seg.view(np.uint8).reshape(n_iters // GIN, GIN * CHUNK, 4)
        bits = np.unpackbits(byt, axis=-1, bitorder="little")
        # (tt, n, k) -> (tt, k, n)
        pl = bits.transpose(0, 2, 1)
        pl3 = np.concatenate([pl, pl, pl], axis=1).astype(ml_dtypes.bfloat16)
        out.append(np.ascontiguousarray(pl3))
    return out


def combine_output(o, n_iters: int = N_ITERS):
    """[n_iters//GOUT, 128, GOUT*32] f32 -> [per] int64.

    col 32*ts + 2*c + half: lo/hi 13-bit halves of chunk c, iter
    GOUT*tt + ts; addr = CHUNK*t + CH*c + p.
    """
    arr = np.asarray(o, dtype=np.int64).reshape(
        n_iters // GOUT, 128, GOUT, NCH, 2)
    lo = arr[..., 0]                     # [tt, p, ts, c]
    hi = arr[..., 1]
    val = lo + 8192 * hi                 # [tt, p, ts, c]
    return val.transpose(0, 2, 3, 1).reshape(-1)


_NC_CACHE = {}
TRACE = False
LAST_RES = None


def kernel(virtual_addr, W1, b1, W2, b2):
    global LAST_RES
    if "nc" not in _NC_CACHE:
        nc = build_nc(N_ITERS)
        nc.finalize()
        _NC_CACHE["nc"] = nc
    nc = _NC_CACHE["nc"]

    consts = make_const_inputs(W1, b1, W2, b2)
    planes = make_bit_planes(virtual_addr, N_ITERS)
    in_maps = [{"bp": planes[c], **consts} for c in range(NCORES)]

    res = bass_utils.run_bass_kernel_spmd(
        nc, in_maps, list(range(NCORES)), trace=TRACE
    )
    LAST_RES = res

    outs = [combine_output(res.results[c]["outp"]) for c in range(NCORES)]
    return np.concatenate(outs)


# revision 24
# speedup vs baseline: 1.5169x; 1.2004x over previous
"""NeuralMMU Trainium2 kernel (v2: ACT-bound pipeline).

Per core: 131072 addrs, 64 iterations x 2048 addrs.

Engine plan per iteration t (steady state, ~1.9us period):
  ACT   Gelu(+b1): hpre slot(t) PSUM [128,2048] -> h(t) SBUF f32.
        One op per iter; this is the bottleneck engine (~1892 ns).
  PE    L2(t-1): 16 matmuls with SWAPPED operands: stationary lhsT =
        h(t-1)[:, 128c:128c+128] (f32, exact), moving rhs = W2 [128,26]
        f32 -> batch-major logits [128 batch, 26] written into the TAIL
        416 f32 of psum slot(t-1) (bank 3), which gelu(t-1) has already
        consumed.  26 cols * 4 cyc/row * 16 = 1664 cyc.
        L1(t+1): 4 bf16 matmuls k=96 (3-way bf16 split of W1, exact to
        ~2^-27) from host-prepared bf16 bit planes -> slot(t+1).
        Blocks g=0..2 issue early; block g=3 (tail bank) waits until the
        DVE threshold has read slot(t+1)'s previous logits.
  DVE   TT is_gt vs per-logit threshold vector (0.5 - b2[j], f32,
        partition-broadcast) -> bits bf16; TT mult by 2^(j%13) weight
        vector; tensor_reduce sum [128,16,2,13] -> packed lo/hi
        [128,32] f32 into an 8-iter accumulator.
  DMA   in: [96,4096] bf16 planes per 2 iters; out: [128,256] f32 per
        8 iters.  Host packs bit planes and combines lo+8192*hi.

PSUM: exactly 8 banks = 2 slots x [128,2048] f32; L2 output aliases the
tail of the slot (time-multiplexed with hpre data).

Numerics are f32-exact end-to-end except the 3-way-bf16 W1 split
(~2^-27) and the ACT Gelu LUT, identical to the f32 baseline (1/1M
mismatch there).
"""

import numpy as np
from contextlib import ExitStack

import concourse.bass as bass
import concourse.mybir as mybir
import concourse.tile as tile
from concourse import bacc, bass_utils

B = 1_048_576
NCORES = 8
PER = B // NCORES          # 131072 addrs per core
BLK = 512                  # addrs per L1 PE block
NBLK = 4                   # L1 blocks per iteration
CH = 128                   # addrs per L2 chunk (stationary width)
NCH = 16                   # L2 chunks per iteration
CHUNK = NBLK * BLK         # 2048 addrs per iteration
N_ITERS = PER // CHUNK     # 64
GIN = 2                    # iters per input DMA
GOUT = 8                   # iters per output DMA
NLOG = 26                  # logits per addr
LW = NCH * NLOG            # 416 logit cols per iteration
HLW = LW // 2              # 208: one thresh half

F32 = mybir.dt.float32
BF16 = mybir.dt.bfloat16
AF = mybir.ActivationFunctionType
ALU = mybir.AluOpType

# cst columns (f32): w1b 0:64 (bf16x128), b1c 64:65, w2f 65:91,
# wvec 91:299 (bf16 x416), thvec 299:715
CW1, CB1, CW2, CWV, CTH, CTOT = 0, 64, 65, 91, 299, 715


def build_nc(n_iters: int = N_ITERS, act=AF.Gelu) -> bass.Bass:
    nc = bacc.Bacc("TRN2")
    assert n_iters % GOUT == 0 and n_iters % GIN == 0

    bp = nc.dram_tensor("bp", [n_iters // GIN, 96, GIN * CHUNK], BF16,
                        kind="ExternalInput")
    cst_d = nc.dram_tensor("cst", [128, CTOT], F32, kind="ExternalInput")
    outp = nc.dram_tensor("outp", [n_iters // GOUT, 128, GOUT * 32], F32,
                          kind="ExternalOutput")

    with ExitStack() as ctx:
        tc = ctx.enter_context(tile.TileContext(nc))
        const = ctx.enter_context(tc.tile_pool(name="const", bufs=1))
        rpool = ctx.enter_context(tc.tile_pool(name="rp", bufs=3))
        ppool = ctx.enter_context(
            tc.tile_pool(name="ppool", bufs=1, space="PSUM"))
        hp = ctx.enter_context(tc.tile_pool(name="hp", bufs=2))
        bop = ctx.enter_context(tc.tile_pool(name="bop", bufs=2))
        bwp = ctx.enter_context(tc.tile_pool(name="bwp", bufs=2))
        pksp = ctx.enter_context(tc.tile_pool(name="pksp", bufs=2))

        # One persistent 8-bank psum tensor; all deps are subtile
        # (range-based), avoiding tile-granular pool-rotation WAR chains.
        PP = ppool.tile([128, 2 * CHUNK], F32, name="PP")

        cst = const.tile([128, CTOT], F32)
        nc.sync.dma_start(cst[:], cst_d[:])
        w1b = cst[:, CW1:CB1].bitcast(BF16)      # [128,128] bf16; rows 0-95
        b1c = cst[:, CB1:CW2]
        w2f = cst[:, CW2:CWV]                    # [128, 26] f32
        wv = cst[:, CWV:CTH].bitcast(BF16)       # [128, 416] bf16
        thv = cst[:, CTH:CTOT]                   # [128, 416] f32

        R = {}
        hs = {}
        bos = {}
        pks = None

        def half(t):
            return CHUNK * (t % 2)

        def load_input(g):
            if g < n_iters // GIN and g not in R:
                r = rpool.tile([96, GIN * CHUNK], BF16, name="r")
                nc.sync.dma_start(r[:], bp[g])
                R[g] = r

        def l1(t, blocks):
            """blocks: bank indices 0..3 (512 cols each).  Banks 0 and 1
            hold the previous era's logits in their head cols, so each is
            issued after its thresh (A / B) has read them; banks 2-3 are
            free early."""
            if t >= n_iters:
                return
            r = R[t // GIN]
            for g in blocks:
                c0 = CHUNK * (t % GIN) + BLK * g
                nc.tensor.matmul(
                    PP[:, half(t) + BLK * g:half(t) + BLK * (g + 1)],
                    w1b[0:96, :],
                    r[0:96, c0:c0 + BLK],
                    start=True, stop=True, tile_position=(0, 0),
                )

        def gelu(t):
            h = hp.tile([128, CHUNK], F32, name="h")
            nc.scalar.activation(h[:], PP[:, half(t):half(t) + CHUNK], act,
                                 bias=b1c, scale=1.0)
            hs[t] = h

        NA = 6                     # chunks in bank-0 logit home
        AW = NA * NLOG             # 156 cols
        BW = LW - AW               # 260 cols (chunks 6-15 in bank-1 home)

        def l2col(c):
            """psum col of chunk c's logits: bank0 head for c<NA, bank1
            head (col 512+) for c>=NA."""
            return NLOG * c if c < NA else BLK + NLOG * (c - NA)

        def l2(t, chunks):
            h = hs[t]
            for c in chunks:
                o = l2col(c)
                nc.tensor.matmul(
                    PP[:, half(t) + o:half(t) + o + NLOG],
                    h[:, CH * c:CH * (c + 1)],
                    w2f[:],
                    start=True, stop=True, tile_position=(0, 0),
                )

        def thresh(t, hi):
            """hi=0: bank0 logits (chunks 0..NA-1); hi=1: bank1 logits."""
            if not hi:
                bos[t] = bop.tile([128, LW], BF16, name="bo")
                nc.vector.tensor_tensor(
                    bos[t][:, 0:AW], PP[:, half(t):half(t) + AW],
                    thv[:, 0:AW], op=ALU.is_gt)
            else:
                nc.vector.tensor_tensor(
                    bos[t][:, AW:LW], PP[:, half(t) + BLK:half(t) + BLK + BW],
                    thv[:, 0:BW], op=ALU.is_gt)

        def pack(t):
            nonlocal pks
            bw = bwp.tile([128, LW], BF16, name="bw")
            nc.vector.tensor_tensor(bw[:], bos.pop(t)[:], wv, op=ALU.mult)
            if t % GOUT == 0:
                pks = pksp.tile([128, GOUT * 32], F32, name="pks")
            nc.vector.tensor_reduce(
                pks[:, 32 * (t % GOUT):32 * (t % GOUT + 1)],
                bw[:].rearrange("p (g x) -> p g x", x=13),
                axis=mybir.AxisListType.X,
                op=ALU.add,
            )
            if t % GOUT == GOUT - 1:
                nc.sync.dma_start(outp[t // GOUT], pks[:])

        # Prologue: planes for iters 0-3, L1(0).
        load_input(0)
        load_input(1)
        l1(0, (2, 3, 0, 1))

        for t in range(n_iters):
            gelu(t)
            if t >= 1:
                l2(t - 1, range(NA))
                thresh(t - 1, 0)
                l2(t - 1, range(NA, NCH))
                thresh(t - 1, 1)
                hs.pop(t - 1)
            if t % GIN == 0:
                load_input(t // GIN + 2)
            l1(t + 1, (2, 3))
            if t >= 1:
                pack(t - 1)
            l1(t + 1, (0,))
            l1(t + 1, (1,))

        l2(n_iters - 1, range(NCH))
        hs.pop(n_iters - 1)
        thresh(n_iters - 1, 0)
        thresh(n_iters - 1, 1)
        pack(n_iters - 1)

    return nc


def make_const_inputs(W1, b1, W2, b2):
    import ml_dtypes

    w1 = np.ascontiguousarray(W1[0:32, :], dtype=np.float32)
    hi = w1.astype(ml_dtypes.bfloat16)
    mid = (w1 - hi.astype(np.float32)).astype(ml_dtypes.bfloat16)
    lo = (w1 - hi.astype(np.float32) - mid.astype(np.float32)).astype(
        ml_dtypes.bfloat16
    )
    w1b = np.zeros((128, 128), dtype=ml_dtypes.bfloat16)
    w1b[0:32] = hi
    w1b[32:64] = mid
    w1b[64:96] = lo

    cst = np.zeros((128, CTOT), dtype=np.float32)
    cst[:, CW1:CB1] = np.ascontiguousarray(w1b).view(np.float32)
    cst[:, CB1] = np.asarray(b1, dtype=np.float32)
    cst[:, CW2:CWV] = np.asarray(W2[:, :NLOG], dtype=np.float32)
    wvec = np.tile(
        np.concatenate([2.0 ** np.arange(13), 2.0 ** np.arange(13)]), NCH
    ).astype(ml_dtypes.bfloat16)        # [416]
    cst[:, CWV:CTH] = np.ascontiguousarray(wvec).view(np.float32)[None, :]
    thvec = np.tile(0.5 - np.asarray(b2[:NLOG], dtype=np.float32), NCH)
    cst[:, CTH:CTOT] = thvec[None, :]
    return {"cst": cst}


def make_bit_planes(virtual_addr, n_iters: int = N_ITERS):
    """Per-core [n_iters//GIN, 96, GIN*2048] bf16 0/1 bit planes.

    Partition 32s + k (s = 0..2 replication) of DMA group tt, col
    j*2048 + n = bit k of addr (GIN*tt + j)*2048 + n.
    """
    import ml_dtypes

    va32 = np.asarray(virtual_addr).astype(np.uint32)
    per = n_iters * CHUNK
    ncores = va32.size // per
    out = []
    for c in range(ncores):
        seg = va32[c * per:(c + 1) * per]
        byt = seg.view(np.uint8).reshape(n_iters // GIN, GIN * CHUNK, 4)
        bits = np.unpackbits(byt, axis=-1, bitorder="little")
        # (tt, n, k) -> (tt, k, n)
        pl = bits.transpose(0, 2, 1)
        pl3 = np.concatenate([pl, pl, pl], axis=1).astype(ml_dtypes.bfloat16)
        out.append(np.ascontiguousarray(pl3))
    return out


def combine_output(o, n_iters: int = N_ITERS):
    """[n_iters//GOUT, 128, GOUT*32] f32 -> [per] int64.

    col 32*ts + 2*c + half: lo/hi 13-bit halves of chunk c, iter
    GOUT*tt + ts; addr = CHUNK*t + CH*c + p.
    """
    arr = np.asarray(o, dtype=np.int64).reshape(
        n_iters // GOUT, 128, GOUT, NCH, 2)
    lo = arr[..., 0]                     # [tt, p, ts, c]
    hi = arr[..., 1]
    val = lo + 8192 * hi                 # [tt, p, ts, c]
    return val.transpose(0, 2, 3, 1).reshape(-1)


_NC_CACHE = {}
TRACE = False
LAST_RES = None


def kernel(virtual_addr, W1, b1, W2, b2):
    global LAST_RES
    if "nc" not in _NC_CACHE:
        nc = build_nc(N_ITERS)
        nc.finalize()
        _NC_CACHE["nc"] = nc
    nc = _NC_CACHE["nc"]

    consts = make_const_inputs(W1, b1, W2, b2)
    planes = make_bit_planes(virtual_addr, N_ITERS)
    in_maps = [{"bp": planes[c], **consts} for c in range(NCORES)]

    res = bass_utils.run_bass_kernel_spmd(
        nc, in_maps, list(range(NCORES)), trace=TRACE
    )
    LAST_RES = res

    outs = [combine_output(res.results[c]["outp"]) for c in range(NCORES)]
    return np.concatenate(outs)


# revision 37
# speedup vs baseline: 1.5374x; 1.0135x over previous
"""NeuralMMU Trainium2 kernel (v2: ACT-bound pipeline).

Per core: 131072 addrs, 64 iterations x 2048 addrs.

Engine plan per iteration t (steady state, ~1.9us period):
  ACT   Gelu(+b1): hpre slot(t) PSUM [128,2048] -> h(t) SBUF f32.
        One op per iter; this is the bottleneck engine (~1892 ns).
  PE    L2(t-1): 16 matmuls with SWAPPED operands: stationary lhsT =
        h(t-1)[:, 128c:128c+128] (f32, exact), moving rhs = W2 [128,26]
        f32 -> batch-major logits [128 batch, 26] written into the TAIL
        416 f32 of psum slot(t-1) (bank 3), which gelu(t-1) has already
        consumed.  26 cols * 4 cyc/row * 16 = 1664 cyc.
        L1(t+1): 4 bf16 matmuls k=96 (3-way bf16 split of W1, exact to
        ~2^-27) from host-prepared bf16 bit planes -> slot(t+1).
        Blocks g=0..2 issue early; block g=3 (tail bank) waits until the
        DVE threshold has read slot(t+1)'s previous logits.
  DVE   TT is_gt vs per-logit threshold vector (0.5 - b2[j], f32,
        partition-broadcast) -> bits bf16; TT mult by 2^(j%13) weight
        vector; tensor_reduce sum [128,16,2,13] -> packed lo/hi
        [128,32] f32 into an 8-iter accumulator.
  DMA   in: [96,4096] bf16 planes per 2 iters; out: [128,256] f32 per
        8 iters.  Host packs bit planes and combines lo+8192*hi.

PSUM: exactly 8 banks = 2 slots x [128,2048] f32; L2 output aliases the
tail of the slot (time-multiplexed with hpre data).

Numerics are f32-exact end-to-end except the 3-way-bf16 W1 split
(~2^-27) and the ACT Gelu LUT, identical to the f32 baseline (1/1M
mismatch there).
"""

import numpy as np
from contextlib import ExitStack

import concourse.bass as bass
import concourse.mybir as mybir
import concourse.tile as tile
from concourse import bacc, bass_utils

B = 1_048_576
NCORES = 8
PER = B // NCORES          # 131072 addrs per core
BLK = 512                  # addrs per L1 PE block
NBLK = 4                   # L1 blocks per iteration
CH = 128                   # addrs per L2 chunk (stationary width)
NCH = 16                   # L2 chunks per iteration
CHUNK = NBLK * BLK         # 2048 addrs per iteration
N_ITERS = PER // CHUNK     # 64
GIN = 2                    # iters per input DMA
GOUT = 8                   # iters per output DMA
NLOG = 26                  # logits per addr
LW = NCH * NLOG            # 416 logit cols per iteration
NA = 6                     # chunks whose logits live in the bank-0 home

F32 = mybir.dt.float32
BF16 = mybir.dt.bfloat16
AF = mybir.ActivationFunctionType
ALU = mybir.AluOpType

# cst columns (f32): w1b 0:64 (bf16x128), b1c 64:65, w2f 65:91,
# wvec 91:299 (bf16 x416), thvec 299:715
CW1, CB1, CW2, CWV, CTH, CTOT = 0, 64, 65, 91, 299, 715


def build_nc(n_iters: int = N_ITERS, act=AF.Gelu) -> bass.Bass:
    nc = bacc.Bacc("TRN2")
    assert n_iters % GOUT == 0 and n_iters % GIN == 0

    bp = nc.dram_tensor("bp", [n_iters // GIN, 96, GIN * CHUNK], BF16,
                        kind="ExternalInput")
    cst_d = nc.dram_tensor("cst", [128, CTOT], F32, kind="ExternalInput")
    outp = nc.dram_tensor("outp", [n_iters // GOUT, 128, GOUT * 32], F32,
                          kind="ExternalOutput")

    with ExitStack() as ctx:
        tc = ctx.enter_context(tile.TileContext(nc))
        const = ctx.enter_context(tc.tile_pool(name="const", bufs=1))
        rpool = ctx.enter_context(tc.tile_pool(name="rp", bufs=3))
        ppool = ctx.enter_context(
            tc.tile_pool(name="ppool", bufs=1, space="PSUM"))
        hp = ctx.enter_context(tc.tile_pool(name="hp", bufs=2))
        bop = ctx.enter_context(tc.tile_pool(name="bop", bufs=2))
        bwp = ctx.enter_context(tc.tile_pool(name="bwp", bufs=2))
        pksp = ctx.enter_context(tc.tile_pool(name="pksp", bufs=2))

        # One persistent 8-bank psum tensor; all deps are subtile
        # (range-based), avoiding tile-granular pool-rotation WAR chains.
        PP = ppool.tile([128, 2 * CHUNK], F32, name="PP")

        cst = const.tile([128, CTOT], F32)
        nc.sync.dma_start(cst[:], cst_d[:])
        w1b = cst[:, CW1:CB1].bitcast(BF16)      # [128,128] bf16; rows 0-95
        b1c = cst[:, CB1:CW2]
        w2f = cst[:, CW2:CWV]                    # [128, 26] f32
        wv = cst[:, CWV:CTH].bitcast(BF16)       # [128, 416] bf16
        thv = cst[:, CTH:CTOT]                   # [128, 416] f32

        R = {}
        hs = {}
        bos = {}
        pks = None

        def half(t):
            return CHUNK * (t % 2)

        def load_input(g):
            if g < n_iters // GIN and g not in R:
                r = rpool.tile([96, GIN * CHUNK], BF16, name="r")
                nc.gpsimd.dma_start(r[:], bp[g])
                R[g] = r

        def l1seg(t, s0, s1):
            r = R[t // GIN]
            c0 = CHUNK * (t % GIN) + s0
            nc.tensor.matmul(
                PP[:, half(t) + s0:half(t) + s1],
                w1b[0:96, :],
                r[0:96, c0:c0 + (s1 - s0)],
                start=True, stop=True, tile_position=(0, 0),
            )

        def l1(t, segs):
            """segs: (s0, s1) col ranges.  Banks 0/1 hold the previous
            era's logits in their head cols, so those segments are issued
            after their thresh (A / B) has read them."""
            if t >= n_iters:
                return
            for s0, s1 in segs:
                l1seg(t, s0, s1)

        def gelu(t):
            h = hp.tile([128, CHUNK], F32, name="h")
            nc.scalar.activation(h[:], PP[:, half(t):half(t) + CHUNK], act,
                                 bias=b1c, scale=1.0)
            hs[t] = h

        AW = NA * NLOG             # bank-0 home cols
        BW = LW - AW               # bank-1 home cols (chunks NA..15)

        def l2col(c):
            """psum col of chunk c's logits: bank0 head for c<NA, bank1
            head (col 512+) for c>=NA."""
            return NLOG * c if c < NA else BLK + NLOG * (c - NA)

        def l2(t, chunks):
            h = hs[t]
            for c in chunks:
                o = l2col(c)
                nc.tensor.matmul(
                    PP[:, half(t) + o:half(t) + o + NLOG],
                    h[:, CH * c:CH * (c + 1)],
                    w2f[:],
                    start=True, stop=True, tile_position=(0, 0),
                )

        def thresh(t, hi):
            """hi=0: bank0 logits (chunks 0..NA-1); hi=1: bank1 logits."""
            if not hi:
                bos[t] = bop.tile([128, LW], BF16, name="bo")
                nc.vector.tensor_tensor(
                    bos[t][:, 0:AW], PP[:, half(t):half(t) + AW],
                    thv[:, 0:AW], op=ALU.is_gt)
            else:
                nc.vector.tensor_tensor(
                    bos[t][:, AW:LW], PP[:, half(t) + BLK:half(t) + BLK + BW],
                    thv[:, 0:BW], op=ALU.is_gt)

        def pack(t):
            nonlocal pks
            bw = bwp.tile([128, LW], BF16, name="bw")
            nc.vector.tensor_tensor(bw[:], bos.pop(t)[:], wv, op=ALU.mult)
            if t % GOUT == 0:
                pks = pksp.tile([128, GOUT * 32], F32, name="pks")
            nc.vector.tensor_reduce(
                pks[:, 32 * (t % GOUT):32 * (t % GOUT + 1)],
                bw[:].rearrange("p (g x) -> p g x", x=13),
                axis=mybir.AxisListType.X,
                op=ALU.add,
            )
            if t % GOUT == GOUT - 1:
                nc.sync.dma_start(outp[t // GOUT], pks[:])

        # Warm the ACT gelu table and the PE clock p-state during the
        # first input DMAs: a dummy activation triggers the table load,
        # and a run of dummy bf16 matmuls keeps the PE "continuously
        # executing" so l1(0) runs at full clock.
        warm = const.tile([128, BLK], BF16, name="warm")
        nc.gpsimd.memset(warm[:], 0.0)
        warmo = const.tile([128, 1], F32, name="warmo")
        nc.scalar.activation(warmo[:], warm[:, 0:1], act, scale=1.0)
        for _ in range(7):
            nc.tensor.matmul(
                PP[0:1, 0:BLK], warm[0:1, 0:1], warm[0:1, 0:BLK],
                start=True, stop=True, tile_position=(0, 0),
            )

        # Prologue: planes for iters 0-3 (group 0 split so l1(0) can
        # start after the first half lands), L1(0).
        r0 = rpool.tile([96, GIN * CHUNK], BF16, name="r")
        nc.gpsimd.dma_start(r0[:, 0:CHUNK], bp[0, :, 0:CHUNK])
        nc.gpsimd.dma_start(r0[:, CHUNK:GIN * CHUNK], bp[0, :, CHUNK:GIN * CHUNK])
        R[0] = r0
        load_input(1)
        # l1 segment plan: banks 2-3 free; bank0 (holds home A) after
        # threshA; bank1 (home B) after threshB.
        SEG_FREE = [(1024, 1536), (1536, 2048)]
        SEG_A = [(0, BLK)]
        SEG_B = [(BLK, 2 * BLK)]

        l1(0, SEG_FREE + SEG_A + SEG_B)

        for t in range(n_iters):
            gelu(t)
            if t >= 1:
                l2(t - 1, range(NA))
                thresh(t - 1, 0)
                l2(t - 1, range(NA, NCH))
                thresh(t - 1, 1)
                hs.pop(t - 1)
            if t % GIN == 0:
                load_input(t // GIN + 2)
            l1(t + 1, SEG_FREE)
            if t >= 1:
                pack(t - 1)
            l1(t + 1, SEG_A)
            l1(t + 1, SEG_B)

        l2(n_iters - 1, range(NCH))
        hs.pop(n_iters - 1)
        thresh(n_iters - 1, 0)
        thresh(n_iters - 1, 1)
        pack(n_iters - 1)

    return nc


def make_const_inputs(W1, b1, W2, b2):
    import ml_dtypes

    w1 = np.ascontiguousarray(W1[0:32, :], dtype=np.float32)
    hi = w1.astype(ml_dtypes.bfloat16)
    mid = (w1 - hi.astype(np.float32)).astype(ml_dtypes.bfloat16)
    lo = (w1 - hi.astype(np.float32) - mid.astype(np.float32)).astype(
        ml_dtypes.bfloat16
    )
    w1b = np.zeros((128, 128), dtype=ml_dtypes.bfloat16)
    w1b[0:32] = hi
    w1b[32:64] = mid
    w1b[64:96] = lo

    cst = np.zeros((128, CTOT), dtype=np.float32)
    cst[:, CW1:CB1] = np.ascontiguousarray(w1b).view(np.float32)
    cst[:, CB1] = np.asarray(b1, dtype=np.float32)
    cst[:, CW2:CWV] = np.asarray(W2[:, :NLOG], dtype=np.float32)
    wvec = np.tile(
        np.concatenate([2.0 ** np.arange(13), 2.0 ** np.arange(13)]), NCH
    ).astype(ml_dtypes.bfloat16)        # [416]
    cst[:, CWV:CTH] = np.ascontiguousarray(wvec).view(np.float32)[None, :]
    thvec = np.tile(0.5 - np.asarray(b2[:NLOG], dtype=np.float32), NCH)
    cst[:, CTH:CTOT] = thvec[None, :]
    return {"cst": cst}


def make_bit_planes(virtual_addr, n_iters: int = N_ITERS):
    """Per-core [n_iters//GIN, 96, GIN*2048] bf16 0/1 bit planes.

    Partition 32s + k (s = 0..2 replication) of DMA group tt, col
    j*2048 + n = bit k of addr (GIN*tt + j)*2048 + n.
    """
    import ml_dtypes

    va32 = np.asarray(virtual_addr).astype(np.uint32)
    per = n_iters * CHUNK
    ncores = va32.size // per
    out = []
    for c in range(ncores):
        seg = va32[c * per:(c + 1) * per]
        byt = seg.view(np.uint8).reshape(n_iters // GIN, GIN * CHUNK, 4)
        bits = np.unpackbits(byt, axis=-1, bitorder="little")
        # (tt, n, k) -> (tt, k, n)
        pl = bits.transpose(0, 2, 1)
        pl3 = np.concatenate([pl, pl, pl], axis=1).astype(ml_dtypes.bfloat16)
        out.append(np.ascontiguousarray(pl3))
    return out


def combine_output(o, n_iters: int = N_ITERS):
    """[n_iters//GOUT, 128, GOUT*32] f32 -> [per] int64.

    col 32*ts + 2*c + half: lo/hi 13-bit halves of chunk c, iter
    GOUT*tt + ts; addr = CHUNK*t + CH*c + p.
    """
    arr = np.asarray(o, dtype=np.int64).reshape(
        n_iters // GOUT, 128, GOUT, NCH, 2)
    lo = arr[..., 0]                     # [tt, p, ts, c]
    hi = arr[..., 1]
    val = lo + 8192 * hi                 # [tt, p, ts, c]
    return val.transpose(0, 2, 3, 1).reshape(-1)


_NC_CACHE = {}
TRACE = False
LAST_RES = None


def kernel(virtual_addr, W1, b1, W2, b2):
    global LAST_RES
    if "nc" not in _NC_CACHE:
        nc = build_nc(N_ITERS)
        nc.finalize()
        _NC_CACHE["nc"] = nc
    nc = _NC_CACHE["nc"]

    consts = make_const_inputs(W1, b1, W2, b2)
    planes = make_bit_planes(virtual_addr, N_ITERS)
    in_maps = [{"bp": planes[c], **consts} for c in range(NCORES)]

    res = bass_utils.run_bass_kernel_spmd(
        nc, in_maps, list(range(NCORES)), trace=TRACE
    )
    LAST_RES = res

    outs = [combine_output(res.results[c]["outp"]) for c in range(NCORES)]
    return np.concatenate(outs)
